# revision 3
# baseline (speedup 1.0000x reference)
"""Trainium2 Bass kernel for nn_CGP_8899172237465 (gnn_message_passing).

The network is linear in x: a 62x62 operator M = 0.75 I + 0.25 A_norm is
built on the host from the tiny adjacency/GATENet inputs, and

  out[o,v,l] = sum_{t=0..4} sum_c P_t[o,c] (M^t x)[c,v,l] + b[o]

Rather than materializing the 1984x1984 kron operator (16x16 tiles of
512-deep matmuls = tensor-bound), the kernel exploits the (c,w)
factorization with a data-stationary trick that keeps every layout
device-friendly:

Stage A (node mix, all 4 propagated states in one pass): x is stored
  [w, (l,c)] and sliced as the *stationary* operand [128, 128] (two
  62-row chunks stacked per 128 partitions); the moving operand is the
  constant Mcat2 [128, (2,248)] block matrix holding M^1..M^4 columns for
  each slot. One matmul emits Y for two chunks: layout [(l4,c), (t,v)].
Stage B (channel mix): blockdiag_l4(P_t^T) [128,128] stationary, moving
  operand = Y t-slices across an 8-chunk group [128, (j,v)=496], five
  accumulating matmuls per group (t=0 reads a host-pre-transposed x copy).

Outputs leave in device-native layout [(l4,o), (j,v)] and are unscrambled
on the host (free). 8 cores x 4 batches data-parallel.
"""

import numpy as np

V = 62
B, C, L = 32, 32, 512
NST = 5             # states 0..4
N_CORES = 8
BPC = B // N_CORES  # 4

NCHUNK = (L * C) // 128   # 128 chunks/batch; chunk k = l in [4k,4k+4), all c
NGRP = NCHUNK // 8        # 16 groups of 8 chunks
NSLICE = NCHUNK // 2      # 64 stage-A slices (2 chunks each)

_CACHE = {}


def _host_M(adj_PLI, adj_buf, gate_w1, gate_w2):
    a64 = lambda a: np.asarray(a, dtype=np.float64)
    adj_PLI, adj_buf = a64(adj_PLI), a64(adj_buf)
    gate_w1, gate_w2 = a64(gate_w1), a64(gate_w2)
    y = adj_buf @ gate_w1.T
    y = np.where(y > 0, y, np.expm1(y))          # ELU
    y = y @ gate_w2.T
    y = np.maximum(np.tanh(y), 0.0)              # ReLU(Tanh)
    adj = adj_PLI @ y.reshape(V, V) + np.eye(V)
    d_inv = adj.sum(1) ** -0.5
    adj_norm = d_inv[:, None] * adj * d_inv[None, :]
    return 0.75 * np.eye(V) + 0.25 * adj_norm


def _host_weights(adj_PLI, adj_buf, gate_w1, gate_w2, mlp_w, mlp_b):
    """mcat2 [128,496], pbd [5,128,128], bias [128,1] (fp16/fp32)."""
    M = _host_M(adj_PLI, adj_buf, gate_w1, gate_w2)
    mlp_w = np.asarray(mlp_w, np.float64)
    mcat = np.empty((V, 4 * V))
    Mp = np.eye(V)
    for t in range(1, NST):
        Mp = M @ Mp
        mcat[:, (t - 1) * V:t * V] = Mp.T        # [w, v] = M^t[v, w]
    mcat2 = np.zeros((128, 2, 4 * V))
    mcat2[0:V, 0] = mcat
    mcat2[64:64 + V, 1] = mcat
    mcat2 = mcat2.reshape(128, 2 * 4 * V)

    pbd = np.zeros((NST, 128, 128))
    for t in range(NST):
        P_t = mlp_w[:, t * C:(t + 1) * C]        # [o, c]
        for l4 in range(4):
            pbd[t, l4 * C:(l4 + 1) * C, l4 * C:(l4 + 1) * C] = P_t.T
    bias = np.tile(np.asarray(mlp_b, np.float64), 4)[:, None]
    return (mcat2.astype(np.float16), pbd.astype(np.float16),
            np.ascontiguousarray(bias, np.float32))


def _prep_x(x):
    """x [B,C,V,L] fp32 -> (x2 [B,128,8192], xtr [B,128,7936]) fp16."""
    x = np.asarray(x, np.float32)
    # xf [w, (l,c)]: free idx = l*C + c
    xf = x.transpose(0, 2, 3, 1).reshape(B, V, L * C)
    x2 = np.zeros((B, 128, NSLICE, 128), np.float16)
    xfr = xf.reshape(B, V, NSLICE, 2, 128)
    x2[:, 0:V] = xfr[:, :, :, 0]
    x2[:, 64:64 + V] = xfr[:, :, :, 1]
    x2 = x2.reshape(B, 128, NSLICE * 128)
    # xtr [(l4,c), (k,w)]: xtr[n, l4*C+c, k*62+w] = x[n, c, w, 4k+l4]
    xt = x.transpose(0, 3, 1, 2).reshape(B, NCHUNK, 4, C, V)  # [n,k,l4,c,w]
    xtr = np.ascontiguousarray(
        xt.transpose(0, 2, 3, 1, 4)            # [n, l4, c, k, w]
        .reshape(B, 128, NCHUNK * V)).astype(np.float16)
    return x2, xtr


def _unscramble(dev):
    """dev [BPC,16,128,496] fp16 -> [BPC, C, V, L] fp32."""
    d = dev.astype(np.float32).reshape(BPC, NGRP, 4, C, 8, V)  # n,g,l4,o,j,v
    return np.ascontiguousarray(
        d.transpose(0, 3, 5, 1, 4, 2)).reshape(BPC, C, V, L)


def _build_program(reps=1):
    from contextlib import ExitStack
    from concourse import bacc, tile, mybir

    nc = bacc.Bacc("TRN2", target_bir_lowering=False, debug=False,
                   enable_asserts=True, num_devices=N_CORES)
    f16, f32 = mybir.dt.float16, mybir.dt.float32
    ID = mybir.ActivationFunctionType.Identity

    x2_ap = nc.dram_tensor("x2", [BPC, 128, NSLICE * 128], f16,
                           kind="ExternalInput").ap()
    xtr_ap = nc.dram_tensor("xtr", [BPC, 128, NCHUNK * V], f16,
                            kind="ExternalInput").ap()
    mc_ap = nc.dram_tensor("mcat2", [128, 2 * 4 * V], f16,
                           kind="ExternalInput").ap()
    pb_ap = nc.dram_tensor("pbd", [NST, 128, 128], f16,
                           kind="ExternalInput").ap()
    b_ap = nc.dram_tensor("bias", [128, 1], f32, kind="ExternalInput").ap()
    o_ap = nc.dram_tensor("out", [BPC, NGRP, 128, 8 * V], f16,
                          kind="ExternalOutput").ap()

    with tile.TileContext(nc) as tc, ExitStack() as ctx:
        wpool = ctx.enter_context(tc.tile_pool(name="w", bufs=1))
        xpool = ctx.enter_context(tc.tile_pool(name="x", bufs=2))
        ypool = ctx.enter_context(tc.tile_pool(name="y", bufs=3))
        opool = ctx.enter_context(tc.tile_pool(name="o", bufs=4))
        psa = ctx.enter_context(tc.tile_pool(name="psa", bufs=5, space="PSUM"))
        psb = ctx.enter_context(tc.tile_pool(name="psb", bufs=3, space="PSUM"))

        mc_sb = wpool.tile([128, 2, 4 * V], f16)
        nc.sync.dma_start(mc_sb[:], mc_ap[:])
        pb_sb = [wpool.tile([128, 128], f16, name=f"p{t}") for t in range(NST)]
        for t in range(NST):
            nc.sync.dma_start(pb_sb[t][:], pb_ap[t])
        b_sb = wpool.tile([128, 1], f32)
        nc.sync.dma_start(b_sb[:], b_ap[:])

        def stage_a(x2_sb, g):
            """4 slice-pair matmuls + evacs -> ys tile [(j,t,v)]."""
            ys = ypool.tile([128, 8, 4, V], f16, name="ys", tag="ys")
            for s in range(4):
                ps = psa.tile([128, 2, 4, V], f32, name="psa", tag="psa")
                sl = g * 4 + s
                nc.tensor.matmul(ps[:],
                                 x2_sb[:, sl * 128:(sl + 1) * 128],
                                 mc_sb[:], start=True, stop=True)
                dst = ys[:, 2 * s:2 * s + 2]
                if s % 2 == 0:
                    nc.scalar.activation(dst, ps[:], ID)
                else:
                    nc.vector.tensor_copy(dst, ps[:])
            return ys

        def stage_b(n, g, ys, xtr_sb):
            pso = psb.tile([128, 8, V], f32, name="pso", tag="pso")
            for t in range(NST):
                if t == 0:
                    rhs = xtr_sb[:, g * 8 * V:(g + 1) * 8 * V]
                else:
                    rhs = ys[:, :, t - 1]
                nc.tensor.matmul(pso[:], pb_sb[t][:], rhs,
                                 start=(t == 0), stop=(t == NST - 1),
                                 skip_group_check=True)
            ob = opool.tile([128, 8 * V], f16, name="ob", tag="ob")
            nc.scalar.activation(ob[:], pso[:], ID, bias=b_sb[:, 0:1])
            nc.scalar.dma_start(o_ap[n, g], ob[:])

        def body():
            for n in range(BPC):
                x2_sb = xpool.tile([128, NSLICE * 128], f16, name="x2", tag="x2")
                nc.sync.dma_start(x2_sb[:], x2_ap[n])
                xtr_sb = xpool.tile([128, NCHUNK * V], f16, name="xtr", tag="xtr")
                nc.sync.dma_start(xtr_sb[:], xtr_ap[n])
                # software pipeline: stage A runs one group ahead of stage B
                ys_prev = stage_a(x2_sb, 0)
                for g in range(NGRP):
                    ys_next = stage_a(x2_sb, g + 1) if g + 1 < NGRP else None
                    stage_b(n, g, ys_prev, xtr_sb)
                    ys_prev = ys_next

        if reps == 1:
            body()
        else:
            with tc.For_i(0, reps, 1):
                body()

    nc.compile()
    return nc


def _in_maps(inputs):
    mcat2, pbd, bias = _host_weights(
        inputs["adj_PLI"], inputs["adj_buf"], inputs["gate_w1"],
        inputs["gate_w2"], inputs["mlp_w"], inputs["mlp_b"])
    x2, xtr = _prep_x(inputs["x"])
    return [
        {"x2": np.ascontiguousarray(x2[i * BPC:(i + 1) * BPC]),
         "xtr": np.ascontiguousarray(xtr[i * BPC:(i + 1) * BPC]),
         "mcat2": mcat2, "pbd": pbd, "bias": bias}
        for i in range(N_CORES)
    ]


def kernel(x, adj_PLI, adj_buf, gate_w1, gate_w2, mlp_w, mlp_b):
    from concourse.bass_utils import run_bass_kernel_spmd

    in_maps = _in_maps(dict(x=x, adj_PLI=adj_PLI, adj_buf=adj_buf,
                            gate_w1=gate_w1, gate_w2=gate_w2,
                            mlp_w=mlp_w, mlp_b=mlp_b))
    if "nc" not in _CACHE:
        _CACHE["nc"] = _build_program()
    nc = _CACHE["nc"]

    res = run_bass_kernel_spmd(nc, in_maps, list(range(N_CORES)))
    if res.exec_time_ns is not None:
        print(f"HW exec time: {res.exec_time_ns} ns")

    out = np.empty((B, C, V, L), dtype=np.float32)
    for i in range(N_CORES):
        out[i * BPC:(i + 1) * BPC] = _unscramble(res.results[i]["out"])
    return out


# revision 11
# speedup vs baseline: 6.4748x; 6.4748x over previous
"""Trainium2 Bass kernel for nn_CGP_8899172237465 (gnn_message_passing).

The network is linear in x: a 62x62 operator M = 0.75 I + 0.25 A_norm is
built on the host from the tiny adjacency/GATENet inputs, and

  out[o,v,l] = sum_{t=0..4} sum_c P_t[o,c] (M^t x)[c,v,l] + b[o]

Rather than materializing the 1984x1984 kron operator (16x16 tiles of
512-deep matmuls = tensor-bound), the kernel exploits the (c,w)
factorization with a data-stationary trick that keeps every layout
device-friendly:

Stage A (node mix, all 4 propagated states in one pass): x is stored
  [w, (l,c)] and sliced as the *stationary* operand [128, 128] (two
  62-row chunks stacked per 128 partitions); the moving operand is the
  constant Mcat2 [128, (2,248)] block matrix holding M^1..M^4 columns for
  each slot. One matmul emits Y for two chunks: layout [(l4,c), (t,v)].
Stage B (channel mix): blockdiag_l4(P_t^T) [128,128] stationary, moving
  operand = Y t-slices across an 8-chunk group [128, (j,v)=496], five
  accumulating matmuls per group (t=0 reads a host-pre-transposed x copy).

Outputs leave in device-native layout [(l4,o), (j,v)] and are unscrambled
on the host (free). 8 cores x 4 batches data-parallel.
"""

import numpy as np

V = 62
B, C, L = 32, 32, 512
NST = 5             # states 0..4
N_CORES = 8
BPC = B // N_CORES  # 4

NCHUNK = (L * C) // 128   # 128 chunks/batch; chunk k = l in [4k,4k+4), all c
NGRP = NCHUNK // 8        # 16 groups of 8 chunks
NSLICE = NCHUNK // 2      # 64 stage-A slices (2 chunks each)

_CACHE = {}


def _host_M(adj_PLI, adj_buf, gate_w1, gate_w2):
    a64 = lambda a: np.asarray(a, dtype=np.float64)
    adj_PLI, adj_buf = a64(adj_PLI), a64(adj_buf)
    gate_w1, gate_w2 = a64(gate_w1), a64(gate_w2)
    y = adj_buf @ gate_w1.T
    y = np.where(y > 0, y, np.expm1(y))          # ELU
    y = y @ gate_w2.T
    y = np.maximum(np.tanh(y), 0.0)              # ReLU(Tanh)
    adj = adj_PLI @ y.reshape(V, V) + np.eye(V)
    d_inv = adj.sum(1) ** -0.5
    adj_norm = d_inv[:, None] * adj * d_inv[None, :]
    return 0.75 * np.eye(V) + 0.25 * adj_norm


def _host_weights(adj_PLI, adj_buf, gate_w1, gate_w2, mlp_w, mlp_b):
    """mcat2 [128,496], pbd [5,128,128], bias [128,1] (fp16/fp32)."""
    M = _host_M(adj_PLI, adj_buf, gate_w1, gate_w2)
    mlp_w = np.asarray(mlp_w, np.float64)
    mcat = np.empty((V, 4 * V))
    Mp = np.eye(V)
    for t in range(1, NST):
        Mp = M @ Mp
        mcat[:, (t - 1) * V:t * V] = Mp.T        # [w, v] = M^t[v, w]
    mcat2 = np.zeros((128, 2, 4 * V))
    mcat2[0:V, 0] = mcat
    mcat2[64:64 + V, 1] = mcat
    mcat2 = mcat2.reshape(128, 2 * 4 * V)

    pbd = np.zeros((NST, 128, 128))
    for t in range(NST):
        P_t = mlp_w[:, t * C:(t + 1) * C]        # [o, c]
        for l4 in range(4):
            pbd[t, l4 * C:(l4 + 1) * C, l4 * C:(l4 + 1) * C] = P_t.T
    bias = np.tile(np.asarray(mlp_b, np.float64), 4)[:, None]
    return (mcat2.astype(np.float16), pbd.astype(np.float16),
            np.ascontiguousarray(bias, np.float32))


def _prep_x(x):
    """x [B,C,V,L] fp32 -> (x2 [B,128,8192], xtr [B,128,7936]) fp16."""
    x = np.asarray(x, np.float32)
    # xf [w, (l,c)]: free idx = l*C + c
    xf = x.transpose(0, 2, 3, 1).reshape(B, V, L * C)
    x2 = np.zeros((B, 128, NSLICE, 128), np.float16)
    xfr = xf.reshape(B, V, NSLICE, 2, 128)
    x2[:, 0:V] = xfr[:, :, :, 0]
    x2[:, 64:64 + V] = xfr[:, :, :, 1]
    x2 = x2.reshape(B, 128, NSLICE * 128)
    # xtr [(l4,c), (k,w)]: xtr[n, l4*C+c, k*62+w] = x[n, c, w, 4k+l4]
    xt = x.transpose(0, 3, 1, 2).reshape(B, NCHUNK, 4, C, V)  # [n,k,l4,c,w]
    xtr = np.ascontiguousarray(
        xt.transpose(0, 2, 3, 1, 4)            # [n, l4, c, k, w]
        .reshape(B, 128, NCHUNK * V)).astype(np.float16)
    return x2, xtr


def _unscramble(dev):
    """dev [BPC,16,128,496] fp16 -> [BPC, C, V, L] fp32."""
    d = dev.astype(np.float32).reshape(BPC, NGRP, 4, C, 8, V)  # n,g,l4,o,j,v
    return np.ascontiguousarray(
        d.transpose(0, 3, 5, 1, 4, 2)).reshape(BPC, C, V, L)


def _build_program(reps=1):
    from contextlib import ExitStack
    from concourse import bacc, tile, mybir

    nc = bacc.Bacc("TRN2", target_bir_lowering=False, debug=False,
                   enable_asserts=True, num_devices=N_CORES)
    f16, f32 = mybir.dt.float16, mybir.dt.float32
    ID = mybir.ActivationFunctionType.Identity

    x2_ap = nc.dram_tensor("x2", [BPC, 128, NSLICE * 128], f16,
                           kind="ExternalInput").ap()
    xtr_ap = nc.dram_tensor("xtr", [BPC, 128, NCHUNK * V], f16,
                            kind="ExternalInput").ap()
    mc_ap = nc.dram_tensor("mcat2", [128, 2 * 4 * V], f16,
                           kind="ExternalInput").ap()
    pb_ap = nc.dram_tensor("pbd", [NST, 128, 128], f16,
                           kind="ExternalInput").ap()
    b_ap = nc.dram_tensor("bias", [128, 1], f32, kind="ExternalInput").ap()
    o_ap = nc.dram_tensor("out", [BPC, NGRP, 128, 8 * V], f16,
                          kind="ExternalOutput").ap()

    with tile.TileContext(nc) as tc, ExitStack() as ctx:
        wpool = ctx.enter_context(tc.tile_pool(name="w", bufs=1))
        xpool = ctx.enter_context(tc.tile_pool(name="x", bufs=2))
        ypool = ctx.enter_context(tc.tile_pool(name="y", bufs=3))
        opool = ctx.enter_context(tc.tile_pool(name="o", bufs=4))
        psa = ctx.enter_context(tc.tile_pool(name="psa", bufs=5, space="PSUM"))
        psb = ctx.enter_context(tc.tile_pool(name="psb", bufs=3, space="PSUM"))

        mc_sb = wpool.tile([128, 2, 4 * V], f16)
        nc.sync.dma_start(mc_sb[:], mc_ap[:])
        pb_sb = [wpool.tile([128, 128], f16, name=f"p{t}") for t in range(NST)]
        for t in range(NST):
            nc.sync.dma_start(pb_sb[t][:], pb_ap[t])
        b_sb = wpool.tile([128, 1], f32)
        nc.sync.dma_start(b_sb[:], b_ap[:])

        def stage_a(x2_sb, g):
            """4 slice-pair matmuls + evacs -> ys tile [(j,t,v)]."""
            ys = ypool.tile([128, 8, 4, V], f16, name="ys", tag="ys")
            for s in range(4):
                ps = psa.tile([128, 2, 4, V], f32, name="psa", tag="psa")
                sl = g * 4 + s
                nc.tensor.matmul(ps[:],
                                 x2_sb[:, sl * 128:(sl + 1) * 128],
                                 mc_sb[:], start=True, stop=True)
                # evacuate PSUM -> SBUF fp16, alternating ACT / DVE
                dst = ys[:, 2 * s:2 * s + 2]
                if s % 2 == 0:
                    nc.scalar.activation(dst, ps[:], ID)
                else:
                    nc.vector.tensor_copy(dst, ps[:])
            return ys

        def stage_b_t0(n, g, xtr_sb):
            pso = psb.tile([128, 8, V], f32, name="pso", tag="pso")
            nc.tensor.matmul(pso[:], pb_sb[0][:],
                             xtr_sb[:, g * 8 * V:(g + 1) * 8 * V],
                             start=True, stop=False, skip_group_check=True)
            return pso

        def stage_b(n, g, ys, pso):
            for t in range(1, NST):
                nc.tensor.matmul(pso[:], pb_sb[t][:], ys[:, :, t - 1],
                                 start=False, stop=(t == NST - 1),
                                 skip_group_check=True)
            ob = opool.tile([128, 8 * V], f16, name="ob", tag="ob")
            nc.scalar.activation(ob[:], pso[:], ID, bias=b_sb[:, 0:1])
            # store from the SP ring: keeps the 632ns/issue HWDGE cost off ACT
            nc.sync.dma_start(o_ap[n, g], ob[:])

        def body():
            NSUB = 8   # split x loads so the first matmuls wait on 1/8th
            for n in range(BPC):
                x2_sb = xpool.tile([128, NSLICE * 128], f16, name="x2", tag="x2")
                xtr_sb = xpool.tile([128, NCHUNK * V], f16, name="xtr", tag="xtr")
                c2, ct = NSLICE * 128 // NSUB, NCHUNK * V // NSUB
                for u in range(NSUB):
                    nc.sync.dma_start(x2_sb[:, u * c2:(u + 1) * c2],
                                      x2_ap[n, :, u * c2:(u + 1) * c2])
                    nc.sync.dma_start(xtr_sb[:, u * ct:(u + 1) * ct],
                                      xtr_ap[n, :, u * ct:(u + 1) * ct])
                # software pipeline: stage A runs one group ahead; stage B's
                # t=0 (no evac dependency) issues before the next stage A
                ys_prev = stage_a(x2_sb, 0)
                for g in range(NGRP):
                    pso = stage_b_t0(n, g, xtr_sb)
                    ys_next = stage_a(x2_sb, g + 1) if g + 1 < NGRP else None
                    stage_b(n, g, ys_prev, pso)
                    ys_prev = ys_next

        if reps == 1:
            body()
        else:
            with tc.For_i(0, reps, 1):
                body()

    nc.compile()
    return nc


def _in_maps(inputs):
    mcat2, pbd, bias = _host_weights(
        inputs["adj_PLI"], inputs["adj_buf"], inputs["gate_w1"],
        inputs["gate_w2"], inputs["mlp_w"], inputs["mlp_b"])
    x2, xtr = _prep_x(inputs["x"])
    return [
        {"x2": np.ascontiguousarray(x2[i * BPC:(i + 1) * BPC]),
         "xtr": np.ascontiguousarray(xtr[i * BPC:(i + 1) * BPC]),
         "mcat2": mcat2, "pbd": pbd, "bias": bias}
        for i in range(N_CORES)
    ]


def kernel(x, adj_PLI, adj_buf, gate_w1, gate_w2, mlp_w, mlp_b):
    from concourse.bass_utils import run_bass_kernel_spmd

    in_maps = _in_maps(dict(x=x, adj_PLI=adj_PLI, adj_buf=adj_buf,
                            gate_w1=gate_w1, gate_w2=gate_w2,
                            mlp_w=mlp_w, mlp_b=mlp_b))
    if "nc" not in _CACHE:
        _CACHE["nc"] = _build_program()
    nc = _CACHE["nc"]

    res = run_bass_kernel_spmd(nc, in_maps, list(range(N_CORES)))
    if res.exec_time_ns is not None:
        print(f"HW exec time: {res.exec_time_ns} ns")

    out = np.empty((B, C, V, L), dtype=np.float32)
    for i in range(N_CORES):
        out[i * BPC:(i + 1) * BPC] = _unscramble(res.results[i]["out"])
    return out
